# revision 59
# baseline (speedup 1.0000x reference)
"""AttentionRNN (BiDAF-style QA reader) Trainium2 kernel, v2.

Per core (pure data-parallel over batch, 4 of 32 rows per core):
  1. Host gathers embeddings (padded to 384 rows; column 300 is a pad-token
     indicator that the z-gate weight row turns into a +1e4 logit, freezing
     h across padding exactly, replacing any explicit mask tensor).
  2. xp projections for the 4 GRU directions (bf16 PE) written in
     (gate, step, chunk, batch) layout with warmup pad chunks per region so
     every per-round slice is contiguous; pad-chunk x is memset (BIGM in z)
     so warmup-frozen chains freeze via the same z-pin mechanism.
  3. GRU scan, chunked-parallel: payload chunks of 16 (passage) / 8
     (question) steps, W=10 warmup steps re-run from h=0 (warmup error is
     below the bf16 noise floor).  Two software streams (fwd dirs / bwd
     dirs) run half-a-round offset.  Per round, x slices enter PSUM via
     identity-matmul accumulation; whh @ h' is decomposed by linearity into
     whh@u + whh@v (u = n + z*h, v = -z*n) so the PE input is ready one
     elementwise op after tanh; h' materializes off the critical path and
     payload h' writes the encoder slots directly.
  4. Attention: transposed-logit formulation (softmax row constants w1.p
     and attn_b cancel), per-batch exp on [64,P] tiles with normalization
     deferred through the head matmuls (block-sparse ones lhs accumulates
     per-(head,b) column sums into rows 0:8; se = seA + seB * 1/sums),
     heads via block-sparse lhs weights into rows 0:8, log-softmax tail.
     All PE ops keep partition base 0/32/64 (base 96 and cross-base lhs/rhs
     crash the device).
"""

import contextlib

import numpy as np
import ml_dtypes

import concourse.bass as bass
import concourse.mybir as mybir
from concourse.masks import make_identity
from concourse.tile import TileContext
from concourse.bass_utils import run_bass_kernel_spmd

F32 = mybir.dt.float32
BF16 = mybir.dt.bfloat16
U8 = mybir.dt.uint8
AX = mybir.AxisListType.X
ALU = mybir.AluOpType
AF = mybir.ActivationFunctionType

B, P, Q, E, H, VOCAB = 32, 512, 64, 300, 256, 50000
HH = 128
EPAD = 384
E2 = 64  # third-kc-chunk rows, 45 real (dims 256..299 + indicator) padded
E2R = 45  # real rows in the third chunk
NC = 8
BC = B // NC
NEG = -1e7
BIGM = 1.0e4

import os
SP_ = 16
SQ_ = int(os.environ.get("KSQ", "8"))  # q payload chunk size
NCP, NCQ = P // SP_, Q // SQ_   # 32, 8 chunks
W = int(os.environ.get("KW", "7"))  # warmup rounds (contraction ~0.65/step)
PADP = (W + SP_ - 1) // SP_     # warmup pad chunks
PADQ = (W + SQ_ - 1) // SQ_
CPP, CPQ = NCP + PADP, NCQ + PADQ
RND = W + SP_                # total rounds
NTP, NTQ = BC * P, BC * Q    # 2048, 256

# x tile free-dim strides (elements), layout [128, gate(3), s, cpad, b]
XPG, XPS = SP_ * CPP * BC, CPP * BC     # 2176, 136
XQG, XQS = SQ_ * CPQ * BC, CPQ * BC     # 320, 40
# enc layout [128, s(16), c(40), b(4)]: p chunks 0:32, q chunks 32:40
ENCC = (NCP + NCQ) * BC                 # 160 cols per s-row

_CACHE = {}

V_ACCUM = os.environ.get("KV_ACCUM", "1") == "1"   # exp accum_out in lsm
V_INPLACE = os.environ.get("KV_INPLACE", "0") == "1"  # t1 in-place psum + PE xn
V_SIG3D = os.environ.get("KV_SIG3D", "0") == "1"   # single 3D-AP sigmoid
_PH = int(os.environ.get("KPH", "3"))  # 1=projections 2=+scan 3=full



def _build_nc():
    nc = bass.Bass()

    epTp_d = nc.declare_dram_parameter("epTp_d", [128, 2 * NTP], BF16,
                                       isOutput=False)
    epTp2_d = nc.declare_dram_parameter("epTp2_d", [E2, NTP], BF16,
                                        isOutput=False)
    epTq_d = nc.declare_dram_parameter("epTq_d", [128, 2 * NTQ], BF16,
                                       isOutput=False)
    q2_d = nc.declare_dram_parameter("q2pack", [E2, NTQ + 4 * 3 * HH], BF16,
                                     isOutput=False)
    wihT = nc.declare_dram_parameter("wihT", [128, 4 * 2 * 3 * HH], BF16,
                                     isOutput=False)
    browD = nc.declare_dram_parameter("brow", [1, 576 + NTQ], BF16,
                                      isOutput=False)
    attwD = nc.declare_dram_parameter("attwp", [128, 238 + 4 * 3 * HH + 24],
                                      BF16, isOutput=False)
    outw = nc.declare_dram_parameter("outw", [HH, 4], F32, isOutput=False)
    pm4 = nc.declare_dram_parameter("pm4", [BC, P], BF16, isOutput=False)
    out = nc.declare_dram_parameter("out", [4 * BC, P], F32, isOutput=True)

    es = contextlib.ExitStack()

    def sb(name, shape, dtype):
        return es.enter_context(nc.sbuf_tensor(name, shape, dtype))

    # raw sbuf: written only in the pre-Tile preamble
    epTp = sb("epTp", [128, 2 * NTP], BF16)
    epTp2 = sb("epTp2", [E2, NTP], BF16)
    epTq = sb("epTq", [128, 2 * NTQ], BF16)
    q2_sb = sb("q2_sb", [E2, NTQ + 4 * 3 * HH], BF16)
    wih_sb = sb("wih_sb", [128, 4 * 2 * 3 * HH], BF16)
    brow_sb = sb("brow_sb", [1, 576 + NTQ], BF16)
    attw_sb = sb("attw_sb", [128, 238 + 4 * 3 * HH + 24], BF16)
    outw_sb = sb("outw_sb", [128, 4], F32)
    pm4_sb = sb("pm4_sb", [BC, P], BF16)
    ones_sb = sb("ones_sb", [128, 512], BF16)
    ident_sb = sb("ident_sb", [128, 128], BF16)

    crit_sem = es.enter_context(nc.semaphore("crit_sem"))
    eptp_sem = es.enter_context(nc.semaphore("eptp_sem"))
    auxa_sem = es.enter_context(nc.semaphore("auxa_sem"))
    init_sem = es.enter_context(nc.semaphore("init_sem"))
    shcrit_sem = es.enter_context(nc.semaphore("shcrit_sem"))
    sheptp_sem = es.enter_context(nc.semaphore("sheptp_sem"))

    # ---- pre-tc preamble ----
    # In-tc DRAM->SBUF DMAs crash the exec unit on this stack, and every
    # HWDGE dispatch costs ~650ns of SEQ + shared-HWDGE time, so the load
    # count is pruned to 11 and split across the two HWDGE queues: SP
    # carries only the four q-projection inputs, then relays crit_sem into
    # an in-block shadow inc at ~5.5us; Act carries the rest (tiny packs
    # first, then epTp and the late weights).  Bus order doubles as the
    # ordering guarantee for late consumers: whh and the attention pack
    # transfer right after epTp, several us before PE can reach their
    # first consumer (scan round 1 / attention tail).
    # crit: wih=16 epTq01=32 q2pack=48 | eptp: epTp01=16 epTp2=32
    for _dst, _src in (
            (wih_sb[:, :], wihT[:, :]),
            (epTq[:, :], epTq_d[:, :]),
            (q2_sb[:, :], q2_d[:, :])):
        nc.sync.dma_start(out=_dst, in_=_src).then_inc(crit_sem, 16)
    for _dst, _src, _sem in (
            (brow_sb[:, :], browD[:, :], None),
            (pm4_sb[:, :], pm4[:, :], None),
            (epTp[:, :], epTp_d[:, :], eptp_sem),
            (epTp2[:, :], epTp2_d[:, :], eptp_sem),
            (attw_sb[:, :], attwD[:, :], None),
            (outw_sb[0:HH, :], outw[:, :], None)):
        nc.scalar.dma_start(out=_dst, in_=_src).then_inc(
            _sem if _sem is not None else auxa_sem, 16)

    # identity on gpsimd; constants on vector
    nc.gpsimd.memset(ident_sb[:, :], 0.0)
    nc.gpsimd.affine_select(
        out=ident_sb[:, :], in_=ident_sb[:, :],
        compare_op=ALU.not_equal, fill=1.0, base=0,
        pattern=[[-1, 128]], channel_multiplier=1)
    nc.gpsimd.sem_inc(init_sem, 1)
    nc.vector.memset(ones_sb[:, :], 1.0)
    nc.vector.sem_inc(init_sem, 1)

    # Shadow relays: SP / Pool wait pre-tc for the real DMA sems, then
    # fire an IN-BLOCK shadow inc that the tile scheduler can model; PE
    # waits on the shadows in-block so q-proj starts as soon as wih+epTq
    # land while epTp still streams.  PE's other raw reads (bhnr round 0,
    # whh round 1, sew/sew24/ones8 in the attention tail) ride the bus
    # behind epTp and are resident well before PE can reach them.
    nc.tensor.wait_ge(init_sem, 2)    # ident + ones
    nc.sync.wait_ge(crit_sem, 48)     # relay: q-proj inputs
    nc.gpsimd.wait_ge(eptp_sem, 32)   # relay: epTp chunks

    with TileContext(nc) as tc:
        with tc.tile_pool(name="ps", bufs=2, space="PSUM") as ps, \
             tc.tile_pool(name="sb", bufs=2) as sbp, \
             tc.tile_pool(name="pst", bufs=1) as pst:

            nc.sync.sem_inc(shcrit_sem, 1)
            nc.gpsimd.sem_inc(sheptp_sem, 1)
            _anch = {}

            def pt(name, shape, dtype):
                return pst.tile(shape, dtype, name=name, tag=name)

            ident = ident_sb

            # PE pre-warm: hold the tensor engine's pstate at full clock
            # through the projection phase (ramp: 3us continuous -> 2.4GHz)
            warm = ps.tile([128, 512], F32, name="warm", tag="tq", bufs=1)
            for _ in range(12):
                nc.tensor.matmul(warm[:, :], ident[:, :], ones_sb[:, :],
                                 start=True, stop=True)
            _anch["wc"] = nc.tensor.wait_ge(shcrit_sem, 1)

            xPA = pt("xPA", [128, 3, SP_, CPP, BC], BF16)
            xPB = pt("xPB", [128, 3, SP_, CPP, BC], BF16)
            xQA = pt("xQA", [128, 3, SQ_, CPQ, BC], BF16)
            xQB = pt("xQB", [128, 3, SQ_, CPQ, BC], BF16)
            encA = pt("encA", [128, SP_, ENCC // BC, BC], BF16)
            encB = pt("encB", [128, SP_, ENCC // BC, BC], BF16)
            hA = pt("hA", [128, ENCC], BF16)
            hB = pt("hB", [128, ENCC], BF16)

            pencFB = pt("pencFB", [128, 2 * NTP], BF16)
            qencFB = pt("qencFB", [128, 2 * NTQ], BF16)
            qenc3 = pt("qenc3", [128, 2 * NTQ], BF16)
            qencT = pt("qencT", [64, 8 * HH], BF16)
            qwm = pt("qwm", [1, NTQ], BF16)
            rs8_sb = pt("rs8_sb", [2 * BC, P], F32)
            pawFB = pt("pawFB", [128, 2 * NTP], BF16)
            qsw2 = pt("qsw2", [2, NTQ], BF16)
            qswT8 = pt("qswT8", [64, 32], BF16)
            se8 = pt("se8", [2 * BC, P], F32)
            lsm_sb = pt("lsm_sb", [2 * BC, P], F32)
            lse_sb = pt("lse_sb", [2 * BC, P], F32)
            red_sb = pt("red_sb", [2 * BC, 8], F32)

            nc.vector.memset(hA[:, :], 0)
            nc.vector.memset(hB[:, :], 0)
            nc.vector.memset(qswT8[:, :], 0)
            # x pad chunks: r/n gates -> 0 ; z gate -> BIGM (freeze)
            for xt, np_, c0 in ((xPA, PADP, 0), (xPB, PADP, NCP),
                                (xQA, PADQ, 0), (xQB, PADQ, NCQ)):
                spad = xt[:, :, :, c0:c0 + np_, :]
                nc.vector.memset(spad[:, 0, :, :, :], 0)
                nc.vector.memset(spad[:, 2, :, :, :], 0)
                nc.vector.memset(spad[:, 1, :, :, :], BIGM)

            # ---- projections ----
            # p dirs: per (dir, batch, gate): psum [128,512] = sum_kc wih.T@ep
            # (GPSIMD cannot access PSUM, so moves rotate Act/DVE only)
            mveng = [nc.scalar, nc.vector, nc.gpsimd]
            mvi = 0

            def move(dst, src):
                nonlocal mvi
                eng = mveng[mvi % 2]
                mvi += 1
                if eng is nc.scalar:
                    eng.activation(dst, src, AF.Copy)
                else:
                    eng.tensor_scalar_add(dst, src, 0.0)

            # q dirs projected fully upfront (all q steps feed rounds 0-7),
            # kc-major so matmuls start as soon as each input chunk lands
            for di, (xt, c0) in ((2, (xQA, PADQ)), (3, (xQB, 0))):
                pps = [ps.tile([128, 512], F32, name="pj", tag="pj",
                               bufs=3) for _ in range(3)]
                for kc in range(3):
                    for g in range(3):
                        if kc < 2:
                            wcol = ((di * 2 + kc) * 3 + g) * HH
                            lhs = wih_sb[:, wcol:wcol + HH]
                            rhs = epTq[:, kc * NTQ:(kc + 1) * NTQ]
                        else:
                            wcol = NTQ + (di * 3 + g) * HH
                            lhs = q2_sb[:, wcol:wcol + HH]
                            rhs = q2_sb[:, 0:NTQ]
                        _h = nc.tensor.matmul(pps[g][:, 0:NTQ], lhs, rhs,
                                              start=(kc == 0), stop=(kc == 2))
                        if di == 2 and kc == 0 and g == 0:
                            _anch["qproj"] = _h
                for g in range(3):
                    dst = xt[:, g, :, c0:c0 + NCQ, :] \
                        .rearrange("p s c b -> p b c s")
                    move(dst, pps[g][:, 0:NTQ])

            # p dirs as per-2-step jobs in consumption (pincer) order:
            # stream A consumes s=8..15,0..7; B consumes s=7..0,15..8
            def _pjob(di, xt, c0, sb0, gate=False):
                def f():
                    rhs01 = epTp.rearrange("p (k b c s) -> p k b c s",
                                           k=2, b=BC, s=SP_)
                    rhs2 = epTp2.rearrange("p (b c s) -> p b c s",
                                           b=BC, s=SP_)
                    pps = [ps.tile([128, 512], F32, name="pj", tag="pj",
                                   bufs=3) for _ in range(3)]
                    for kc in range(3):
                        for g in range(3):
                            if kc < 2:
                                wcol = ((di * 2 + kc) * 3 + g) * HH
                                lhs = wih_sb[:, wcol:wcol + HH]
                                rhs = rhs01[:, kc, :, :, sb0:sb0 + 2]
                            else:
                                wcol = NTQ + (di * 3 + g) * HH
                                lhs = q2_sb[:, wcol:wcol + HH]
                                rhs = rhs2[:, :, :, sb0:sb0 + 2]
                            _h = nc.tensor.matmul(
                                pps[g][:, 0:2 * NCP * BC], lhs, rhs,
                                start=(kc == 0), stop=(kc == 2))
                            if gate and kc == 0 and g == 0:
                                _anch["pjob"] = _h
                    for g in range(3):
                        dst = xt[:, g, sb0:sb0 + 2, c0:c0 + NCP, :] \
                            .rearrange("p s c b -> p b c s")
                        move(dst, pps[g][:, 0:2 * NCP * BC])
                return f

            _pjobs = []
            a0 = ((SP_ - W) % SP_) // 2 * 2  # A's first block (even-aligned)
            b0 = ((W - 1) // 2) * 2          # B's first block
            for j in range(8):
                sA = (a0 + 2 * j) % SP_      # A consumes s ascending
                sB = (b0 - 2 * j) % SP_      # B consumes 15-s descending
                _pjobs.append(_pjob(0, xPA, PADP, sA, gate=(j == 0)))
                _pjobs.append(_pjob(1, xPB, 0, sB))
            _anch["we"] = nc.tensor.wait_ge(sheptp_sem, 1)
            for _ in range(2):
                _pjobs.pop(0)()

            if _PH < 2:
                nc.gpsimd.dma_start(out[0:16, :], xPA[0:16, 0, 0:4, PADP:PADP + 32, :])
            # ---- GRU scan ----
            # psum bank layout per stream/round: r[0:160) z[160:320) n[320:480)
            OFR, OFZ, OFN = 0, 160, 320
            SPRM = {"A": (xPA, xQA, encA, hA, 0, 2),
                    "B": (xPB, xQB, encB, hB, 1, 3)}
            _stash = {}

            def geom(st, k):
                e = k - W
                s, coff = e % SP_, e // SP_
                sq, coffq = e % SQ_, e // SQ_
                qact = e < SQ_
                wd = ENCC if qact else NCP * BC
                if st == "A":
                    return e, qact, wd, PADP + coff, PADQ + coffq, s, sq, s
                return (e, qact, wd, -coff, -coffq,
                        SP_ - 1 - s, SQ_ - 1 - sq, SP_ - 1 - s)

            def hsrc_of(st, k):
                xp, xq, enc, hcu, dp, dq = SPRM[st]
                e, qact, wd = geom(st, k)[:3]
                if e - 1 < 0:
                    return hcu[:, 0:wd]
                rprev = (e - 1) if st == "A" else (SP_ - e)
                pcc = ENCC // BC if e - 1 < SQ_ else NCP
                return enc[:, rprev, 0:pcc, :].rearrange(
                    "p c b -> p (c b)")[:, 0:wd]

            def first_half(st, k):
                xp, xq, enc, hcu, dp, dq = SPRM[st]
                e, qact, wd, cp0, cq0, sx, sxq, row = geom(st, k)
                pm = ps.tile([128, 512], F32, name="prz" + st, tag=st)
                mm = nc.tensor.matmul
                first = [True]

                def gmm(dst, lhs, rhs, stop=False):
                    mm(dst, lhs, rhs, start=first[0], stop=stop)
                    first[0] = False

                for g, off in ((0, OFR), (1, OFZ)):
                    gmm(pm[:, off:off + NCP * BC], ident[:, :],
                        xp[:, g, sx, cp0:cp0 + NCP, :])
                    if qact:
                        gmm(pm[:, off + NCP * BC:off + wd], ident[:, :],
                            xq[:, g, sxq, cq0:cq0 + NCQ, :])
                # whh @ h'(k-1) via linearity: h' = u + v (u = n + z*h,
                # v = -z*n); whh@u issues as soon as u is ready, whh@v last.
                if k > 0:
                    for src_t, wtb in zip(_uv[st], (attw_sb, attw_sb)):
                        hp = src_t[:, 0:NCP * BC]
                        hq = src_t[:, NCP * BC:wd] if qact else None
                        for g, off in ((0, OFR), (1, OFZ), (2, OFN)):
                            wc = 238 + (dp * 3 + g) * HH
                            gmm(pm[:, off:off + NCP * BC],
                                wtb[:, wc:wc + HH], hp)
                            if qact:
                                wcq = 238 + (dq * 3 + g) * HH
                                gmm(pm[:, off + NCP * BC:off + wd],
                                    wtb[:, wcq:wcq + HH], hq)
                gmm(pm[:, OFN:OFN + NCP * BC],
                    brow_sb[0:1, dp * HH:(dp + 1) * HH],
                    ones_sb[0:1, 0:NCP * BC], stop=not qact)
                if qact:
                    gmm(pm[:, OFN + NCP * BC:OFN + wd],
                        brow_sb[0:1, dq * HH:(dq + 1) * HH],
                        ones_sb[0:1, 0:NCQ * BC], stop=True)

                # sigmoid r|z in one op
                rz = sbp.tile([128, 2, 160], BF16, name="rz" + st,
                              tag="rz" + st)
                nc.scalar.activation(
                    rz[:, :, 0:wd],
                    pm[:, 0:2 * 160].rearrange(
                        "p (g x) -> p g x", x=160)[:, :, 0:wd],
                    AF.Sigmoid)
                # off-chain: zh = z*h'(k-1)
                zh = sbp.tile([128, 160], BF16, name="zh" + st,
                              tag="zh" + st)
                if k > 0:
                    nc.gpsimd.tensor_mul(zh[:, 0:wd], rz[:, 1, 0:wd],
                                         _hh[st][:, 0:wd])
                # t1 = pn * r in place; += xn via PE identity accumulate
                nc.vector.tensor_mul(pm[:, OFN:OFN + wd],
                                     pm[:, OFN:OFN + wd], rz[:, 0, 0:wd])
                mm(pm[:, OFN:OFN + NCP * BC], ident[:, :],
                   xp[:, 2, sx, cp0:cp0 + NCP, :],
                   start=False, stop=False, skip_group_check=True)
                if qact:
                    mm(pm[:, OFN + NCP * BC:OFN + wd], ident[:, :],
                       xq[:, 2, sxq, cq0:cq0 + NCQ, :],
                       start=False, stop=False, skip_group_check=True)
                _stash[st] = (pm, rz, zh)

            def second_half(st, k):
                xp, xq, enc, hcu, dp, dq = SPRM[st]
                e, qact, wd, cp0, cq0, sx, sxq, row = geom(st, k)
                pm, rz, zh = _stash[st]
                nt = sbp.tile([128, 160], BF16, name="nt" + st,
                              tag="nt" + st)
                nc.scalar.activation(nt[:, 0:wd], pm[:, OFN:OFN + wd],
                                     AF.Tanh)
                # v = -z*n (fused negate: whh@v replaces the old whhN@(z*n))
                # then u = n + zh, back-to-back on DVE
                vt = sbp.tile([128, 160], BF16, name="vt" + st,
                              tag="vt" + st)
                nc.vector.scalar_tensor_tensor(
                    vt[:, 0:wd], rz[:, 1, 0:wd], -1.0, nt[:, 0:wd],
                    op0=ALU.mult, op1=ALU.mult)
                ut = sbp.tile([128, 160], BF16, name="ut" + st,
                              tag="ut" + st)
                if k > 0:
                    nc.vector.tensor_add(ut[:, 0:wd], nt[:, 0:wd],
                                         zh[:, 0:wd])
                else:
                    nc.vector.tensor_copy(ut[:, 0:wd], nt[:, 0:wd])
                _uv[st] = (vt, ut)
                # h' = u + v (off-chain: payload emit / next round's zh)
                if e < 0:
                    hdst = sbp.tile([128, 160], BF16, name="hh" + st,
                                    tag="hh" + st)
                    _hh[st] = hdst
                    hdst = hdst[:, 0:wd]
                else:
                    hdst = enc[:, row, 0:wd // BC, :].rearrange(
                        "p c b -> p (c b)")
                    _hh[st] = hdst
                nc.gpsimd.tensor_add(hdst, ut[:, 0:wd], vt[:, 0:wd])

            _uv = {}
            _hh = {}
            pvv = pencFB.rearrange("p (h b c s) -> p h b c s",
                                   h=2, b=BC, s=SP_)
            _rpk = [nc.vector, nc.gpsimd]

            def _repack_row(half, enc, row, eng):
                # one finalized enc s-row -> pencFB (b,t) layout, one op
                src_ = enc[:, row, 0:NCP, :].rearrange("p c b -> p b c")
                eng.tensor_scalar_add(pvv[:, half, :, :, row], src_, 0.0)
            # q-side attention prep, dribbled into the scan tail
            _qprep = []
            if _PH >= 3:
                def _q_repack(b, half, enc, r0, eng):
                    def f():
                        srcq = enc[:, r0:r0 + SQ_, NCP:NCP + NCQ, b] \
                            .rearrange("p s c -> p c s")
                        dstq = qencFB[:, half * NTQ + b * Q:
                                      half * NTQ + (b + 1) * Q]
                        if eng is nc.scalar:
                            eng.activation(dstq, srcq, AF.Copy)
                        else:
                            eng.tensor_scalar_add(dstq, srcq, 0.0)
                    return f

                qeng = [nc.scalar, nc.vector, nc.gpsimd]
                for b in range(BC):
                    for half, enc, r0 in ((0, encA, 0),
                                          (1, encB, SQ_ % SP_)):
                        _qprep.append(_q_repack(
                            b, half, enc, r0, qeng[(b * 2 + half) % 3]))

                def _qenc3(half):
                    def f():
                        nc.scalar.activation(
                            qenc3[:, half * NTQ:(half + 1) * NTQ],
                            qencFB[:, half * NTQ:(half + 1) * NTQ],
                            AF.Copy, scale=outw_sb[:, half:half + 1])
                    return f
                _qprep.append(_qenc3(0))
                _qprep.append(_qenc3(1))

                def _qwm():
                    pq = ps.tile([128, 512], F32, name="pqw", tag="pj",
                                 bufs=3)
                    nc.tensor.matmul(pq[0:1, 0:NTQ], attw_sb[:, 12:13],
                                     qencFB[:, 0:NTQ], start=True, stop=False)
                    nc.tensor.matmul(pq[0:1, 0:NTQ], attw_sb[:, 13:14],
                                     qencFB[:, NTQ:2 * NTQ], start=False,
                                     stop=True)
                    nc.vector.scalar_tensor_tensor(
                        qwm[0:1, :], brow_sb[0:1, 576:576 + NTQ], NEG, pq[0:1, 0:NTQ],
                        op0=ALU.mult, op1=ALU.add)
                _qprep.append(_qwm)

                def _qsw():
                    # qsw[head, (b,q)] = qenc . sw_attw — folds the attw
                    # head terms into a per-q scalar so seB can consume exT
                    # directly (no attw sbuf bounce)
                    pq2 = ps.tile([128, 512], F32, name="pq2", tag="pj",
                                  bufs=3)
                    nc.tensor.matmul(pq2[0:2, 0:NTQ], attw_sb[:, 4:6],
                                     qencFB[:, 0:NTQ], start=True,
                                     stop=False)
                    nc.tensor.matmul(pq2[0:2, 0:NTQ], attw_sb[:, 6:8],
                                     qencFB[:, NTQ:2 * NTQ], start=False,
                                     stop=True)
                    nc.vector.tensor_scalar_add(qsw2[0:2, :],
                                                pq2[0:2, 0:NTQ], 0.0)
                _qprep.append(_qsw)

                def _qswt(b):
                    def f():
                        ptr = ps.tile([128, 512], BF16, name="ptq",
                                      tag="tq", bufs=1)
                        nc.tensor.transpose(ptr[0:Q, 0:2],
                                            qsw2[0:2, b * Q:(b + 1) * Q],
                                            ident[0:2, 0:2])
                        nc.vector.tensor_scalar_add(
                            qswT8[0:Q, 8 * b + b:8 * b + b + 1],
                            ptr[0:Q, 0:1], 0.0)
                        nc.scalar.activation(
                            qswT8[0:Q, 8 * b + BC + b:8 * b + BC + b + 1],
                            ptr[0:Q, 1:2], AF.Copy)
                    return f
                for b in range(BC):
                    _qprep.append(_qswt(b))

                def _qtr(b, half, eng):
                    def f():
                        ptr = ps.tile([128, 512], BF16, name="ptq", tag="tq",
                                      bufs=1)
                        nc.tensor.transpose(
                            ptr[0:Q, 0:HH],
                            qencFB[:, half * NTQ + b * Q:
                                   half * NTQ + (b + 1) * Q],
                            ident[:, :])
                        col = (b * 2 + half) * HH
                        if eng is nc.scalar:
                            eng.activation(qencT[0:Q, col:col + HH],
                                           ptr[0:Q, 0:HH], AF.Copy)
                        else:
                            eng.tensor_scalar_add(qencT[0:Q, col:col + HH],
                                                  ptr[0:Q, 0:HH], 0.0)
                    return f
                for b in range(BC):
                    for half in range(2):
                        _qprep.append(_qtr(b, half,
                                           qeng[(b * 2 + half) % 2]))

            NRND = RND if _PH >= 2 else 0
            for k in range(NRND):
                first_half("A", k)
                if k > 0:
                    second_half("B", k - 1)
                first_half("B", k)
                second_half("A", k)
                if _PH >= 3 and k >= W:
                    _repack_row(0, encA, k - W, _rpk[k % 2])
                if _PH >= 3 and k > W:
                    _repack_row(1, encB, SP_ - 1 - (k - 1 - W),
                                _rpk[(k + 1) % 2])
                if _pjobs:
                    _pjobs.pop(0)()
                if k == 0 and _pjobs:
                    _pjobs.pop(0)()
                if k > W + SQ_:
                    for _ in range(4):
                        if _qprep:
                            _qprep.pop(0)()
            if NRND:
                second_half("B", NRND - 1)
                if _PH >= 3:
                    _repack_row(1, encB, SP_ - 1 - (NRND - 1 - W),
                                nc.vector)
            while _qprep:
                _qprep.pop(0)()

            if _PH < 3:
                nc.gpsimd.dma_start(out[0:16, :], encA[0:16, 0:4, 0:32, :])

            if _PH >= 3:
                # ---- attention ---- (q-side prep ran in the scan tail)
                # logits read enc directly (strided); no repack barrier
                def pv(enc, b):
                    return enc[:, :, 0:NCP, b].rearrange("p s c -> p c s")

                exT = {}
                for b in range(BC):
                    pt_ = ps.tile([128, 512], F32, name="plgT", tag="A")
                    o = pt_[0:Q, :]
                    nc.tensor.matmul(o, qenc3[:, b * Q:(b + 1) * Q],
                                     pv(encA, b), start=True, stop=False)
                    nc.tensor.matmul(o, qenc3[:, NTQ + b * Q:NTQ + (b + 1) * Q],
                                     pv(encB, b), start=False, stop=False)
                    nc.tensor.matmul(o, qwm[0:1, b * Q:(b + 1) * Q],
                                     ones_sb[0:1, 0:P], start=False, stop=True)
                    ex = sbp.tile([64, 512], BF16, name="exT", tag=f"exT{b}",
                                  bufs=1)
                    nc.scalar.activation(ex[0:Q, :], pt_[0:Q, :], AF.Exp)
                    exT[b] = ex

                # seA head group: penc terms (strided enc reads) with the
                # pad-mask matmul second: the first matmul is enc-gated so
                # the psum allocation stays behind the scan's B-stream
                # banks, and the mask then runs in PE slack, not in the
                # congested seB window.
                seA = ps.tile([128, 512], F32, name="seA", tag="B")
                na = 0
                for b in range(BC):
                    for half, enc in ((0, encA), (1, encB)):
                        blk = (b * 6 + half) * 8
                        nc.tensor.matmul(seA[0:2 * BC, :],
                                         attw_sb[:, 14 + blk:14 + blk + 8],
                                         pv(enc, b), start=(na == 0),
                                         stop=False)
                        na += 1
                        if na == 1:
                            # pad mask: -1e7 = -9961472 - 38400 - 128,
                            # all bf16-exact, so masked rows match the
                            # reference bit-for-bit
                            for mc in (1774, 1782, 1790):
                                nc.tensor.matmul(
                                    seA[0:2 * BC, :],
                                    attw_sb[0:BC, mc:mc + 8],
                                    pm4_sb[0:BC, :], start=False,
                                    stop=False)
                nc.tensor.matmul(seA[0:2 * BC, :], brow_sb[0:1, 512:520],
                                 ones_sb[0:1, 0:P], start=False, stop=True)

                # per-(head,b) column sums into psum rows 0:8 (block-sparse ones)
                sm = ps.tile([128, 512], F32, name="sums", tag="pj", bufs=3)
                for b in range(BC):
                    nc.tensor.matmul(sm[0:2 * BC, :],
                                     attw_sb[0:64, 206 + 8 * b:206 + 8 * b + 8],
                                     exT[b][0:Q, :],
                                     start=(b == 0), stop=(b == BC - 1))
                nc.vector.reciprocal(rs8_sb[:, :], sm[0:2 * BC, :])

                # seB: attw head terms via qswT8 @ exT (no attw bounce),
                # then paw terms; attw_un psum feeds the paw mul directly
                seB = ps.tile([128, 512], F32, name="seB", tag="B")
                for b in range(BC):
                    nc.tensor.matmul(seB[0:2 * BC, :],
                                     qswT8[0:Q, 8 * b:8 * b + 8],
                                     exT[b][0:Q, :],
                                     start=(b == 0), stop=False)
                for b in range(BC):
                    for half in range(2):
                        pw = ps.tile([128, 512], F32, name="paw",
                                     tag="A")
                        col = (b * 2 + half) * HH
                        nc.tensor.matmul(pw[:, :],
                                         qencT[0:Q, col:col + HH],
                                         exT[b][0:Q, :],
                                         start=True, stop=True)
                        sl = slice(half * NTP + b * P,
                                   half * NTP + (b + 1) * P)
                        if (b * 2 + half) % 2 == 0:
                            nc.vector.tensor_mul(pawFB[:, sl],
                                                 pencFB[:, sl], pw[:, :])
                        else:
                            # bounce via Act (idle here); bf16 halves the
                            # DVE mul cost
                            aw_ = sbp.tile([128, 512], BF16, name="awb",
                                           tag="awb")
                            nc.scalar.activation(aw_[:, :], pw[:, :],
                                                 AF.Copy)
                            nc.vector.tensor_mul(pawFB[:, sl],
                                                 pencFB[:, sl], aw_[:, :])
                        blk = (b * 6 + 4 + half) * 8
                        nc.tensor.matmul(seB[0:2 * BC, :],
                                         attw_sb[:, 14 + blk:14 + blk + 8],
                                         pawFB[:, sl], start=False,
                                         stop=(b == BC - 1 and half == 1))
                # t8 = seB*rs8 (bf16), accumulated into the seA psum via
                # identity matmul: se8 lives in psum; the sbuf copy for the
                # raw-logits DMA runs off-chain on Act in parallel
                t8 = sbp.tile([2 * BC, P], BF16, name="t8", tag="t8")
                nc.vector.tensor_mul(t8[:, :], seB[0:2 * BC, :], rs8_sb[:, :])
                nc.tensor.matmul(seA[0:2 * BC, :], ident[0:2 * BC, 0:2 * BC],
                                 t8[:, :], start=False, stop=False,
                                 skip_group_check=True)
                sep = seA[0:2 * BC, :]
                nc.scalar.activation(se8[:, :], sep, AF.Copy)
                nc.sync.dma_start(out[0:2 * BC, :], se8[:, :])
                nc.vector.tensor_reduce(red_sb[:, 1:2], sep, AX, ALU.max,
                                        negate=True)
                nc.scalar.activation(lse_sb[:, :], sep, AF.Exp,
                                     bias=red_sb[:, 1:2],
                                     accum_out=red_sb[:, 2:3])
                nc.scalar.activation(red_sb[:, 3:4], red_sb[:, 2:3], AF.Ln)
                nc.vector.tensor_sub(red_sb[:, 4:5], red_sb[:, 3:4],
                                     red_sb[:, 1:2])
                nc.vector.tensor_scalar(out=lsm_sb[:, :], in0=sep,
                                        scalar1=red_sb[:, 4:5], scalar2=None,
                                        op0=ALU.subtract)
                nc.sync.dma_start(out[2 * BC:4 * BC, :], lsm_sb[:, :])

    _pin_wait(nc, _anch["wc"], _anch["qproj"])
    _pin_wait(nc, _anch["we"], _anch["pjob"])
    _split_multiwaits(nc)
    return nc, es


def _pin_wait(nc, wait_h, anchor_h):
    """The tile scheduler reorders raw EventSemaphore waits freely; pin
    each one directly before its anchor matmul (and the Ldweights feeding
    it) so the raw-tensor read it protects stays protected."""
    wait_i = getattr(wait_h, "ins", wait_h)
    anchor_i = getattr(anchor_h, "ins", anchor_h)
    for b in nc.main_func.blocks:
        il = b.instructions
        if wait_i in il and anchor_i in il:
            il.remove(wait_i)
            ia = il.index(anchor_i)
            while ia > 0 and type(il[ia - 1]).__name__ == "InstLdweights":
                ia -= 1
            il.insert(ia, wait_i)
            return
    raise AssertionError("pin_wait: wait/anchor not found in one block")


def _split_multiwaits(nc):
    """HW instruction encodings hold a single semaphore wait; move extra
    waits emitted by Tile onto same-engine NOPs inserted just before."""
    for b in nc.main_func.blocks:
        il = b.instructions
        newlist = []
        for inst in il:
            if type(inst).__name__ == "InstISA":
                # EVENT_SEMAPHORE_RANGE_CLEAR mis-encodes for this walrus
                # build; NRT clears semaphores per execution anyway.
                continue
            si = inst.sync_info
            if si is not None and len(si.on_wait) > 1:
                waits = list(si.on_wait)
                for wx in waits[:-1]:
                    nop = nc.engines[inst.engine].nop(hint="wsplit").ins
                    for bb in nc.main_func.blocks:
                        try:
                            bb.instructions.remove(nop)
                            break
                        except ValueError:
                            pass
                    nop.sync_info = mybir.SyncInfo(on_wait=[wx], on_update=[])
                    newlist.append(nop)
                inst.sync_info = mybir.SyncInfo(on_wait=[waits[-1]],
                                                on_update=list(si.on_update))
            newlist.append(inst)
        il[:] = newlist


def _prep_core(inputs, c):
    bs = slice(c * BC, (c + 1) * BC)
    ptok = np.asarray(inputs["passage"][bs]).astype(np.int64).reshape(-1)
    qtok = np.asarray(inputs["question"][bs]).astype(np.int64).reshape(-1)
    d = {}
    embp = inputs["_embp"]  # [VOCAB, 256 + E2R]
    ep = embp[ptok].T       # [256 + E2R, NTP]
    d["epTp_d"] = np.ascontiguousarray(
        ep[0:256].reshape(2, 128, NTP).transpose(1, 0, 2).reshape(128, -1))
    ep2 = np.zeros((E2, NTP), ep.dtype)
    ep2[0:E2R] = ep[256:256 + E2R]
    ep2[E2R] = 1.0  # bias row: pairs with the brzn row folded into wih2T
    d["epTp2_d"] = ep2
    eq = embp[qtok].T
    d["epTq_d"] = np.ascontiguousarray(
        eq[0:256].reshape(2, 128, NTQ).transpose(1, 0, 2).reshape(128, -1))
    eq2 = np.zeros((E2, NTQ), eq.dtype)
    eq2[0:E2R] = eq[256:256 + E2R]
    eq2[E2R] = 1.0
    d["_epTq2"] = eq2
    qm0 = (qtok == 0).astype(ml_dtypes.bfloat16)
    d["_qm0"] = np.ascontiguousarray(qm0[None, :])
    d["pm4"] = np.ascontiguousarray(
        (ptok == 0).reshape(BC, P).astype(ml_dtypes.bfloat16))
    return d


def _prep_shared(inputs):
    bf = ml_dtypes.bfloat16

    wihT = np.zeros((4, 2, 128, 3 * HH), bf)      # (d, kc01, p, m)
    wih2T = np.zeros((4, E2, 3 * HH), bf)         # (d, p2, m)
    whhT = np.zeros((4, HH, 3 * HH), bf)          # (d, p, m)
    brzn = np.zeros((4, HH, 3), np.float32)  # folded into wih2T row E2R
    bhnr = np.zeros((1, 576), bf)  # packed into brow with per-core qm0
    for di, (pre, dd) in enumerate((("p", "f"), ("p", "b"),
                                    ("q", "f"), ("q", "b"))):
        wih = np.asarray(inputs[f"{pre}_wih_{dd}"], np.float32)
        whh = np.asarray(inputs[f"{pre}_whh_{dd}"], np.float32)
        bih = np.asarray(inputs[f"{pre}_bih_{dd}"], np.float32)
        bhh = np.asarray(inputs[f"{pre}_bhh_{dd}"], np.float32)
        wT = np.zeros((EPAD, 3 * HH), bf)
        wT[:E, :] = wih.T.astype(bf)
        wT[E, HH:2 * HH] = BIGM  # pad-token mask column -> z-gate freeze
        wihT[di] = wT[0:256].reshape(2, 128, 3 * HH)
        wih2T[di, 0:E2R] = wT[256:256 + E2R]
        whhT[di] = whh.T.astype(bf)
        for gg in range(3):
            brzn[di, :, gg] = bih[gg * HH:(gg + 1) * HH] + (
                bhh[gg * HH:(gg + 1) * HH] if gg < 2 else 0)
        bhnr[0, di * HH:(di + 1) * HH] = bhh[2 * HH:].astype(bf)
    for di in range(4):
        for gg in range(3):
            wih2T[di, E2R, gg * HH:(gg + 1) * HH] = brzn[di, :, gg].astype(bf)
    wihT = np.ascontiguousarray(
        wihT.transpose(2, 0, 1, 3).reshape(128, -1))      # (p,(d,kc,m))
    wih2T = np.ascontiguousarray(
        wih2T.transpose(1, 0, 2).reshape(E2, -1))         # (p2,(d,m))
    whhT = np.ascontiguousarray(
        whhT.transpose(1, 0, 2).reshape(128, -1))         # (p,(d,m))

    aw = np.asarray(inputs["attn_w"], np.float32)
    w2, w3 = aw[256:512], aw[512:]
    outw = np.zeros((HH, 4), np.float32)
    outw[:, 0], outw[:, 1] = w3[:128], w3[128:]
    outw[0:BC, 2] = float(np.asarray(inputs["start_b"]))
    outw[BC:2 * BC, 2] = float(np.asarray(inputs["end_b"]))

    sw = np.asarray(inputs["start_w"], np.float32)
    ew = np.asarray(inputs["end_w"], np.float32)
    sew = np.zeros((HH, 14), bf)
    for j in range(6):
        sew[:, 2 * j] = sw[j * 128:(j + 1) * 128].astype(bf)
        sew[:, 2 * j + 1] = ew[j * 128:(j + 1) * 128].astype(bf)
    sew[:, 12] = w2[:128].astype(bf)
    sew[:, 13] = w2[128:].astype(bf)
    sew24 = np.zeros((HH, 192), bf)
    for b in range(BC):
        for j in range(6):
            blk = (b * 6 + j) * 8
            sew24[:, blk + b] = sw[j * 128:(j + 1) * 128].astype(bf)
            sew24[:, blk + BC + b] = ew[j * 128:(j + 1) * 128].astype(bf)
    ones8 = np.zeros((128, 32), bf)
    for b in range(BC):
        ones8[:, 8 * b + b] = 1.0
        ones8[:, 8 * b + BC + b] = 1.0
    bhnr[0, 512:512 + BC] = np.float32(inputs["start_b"]).astype(bf)
    bhnr[0, 516:516 + BC] = np.float32(inputs["end_b"]).astype(bf)
    attwp = np.zeros((128, 238 + 4 * 3 * HH + 24), bf)
    attwp[0:HH, 0:14] = sew
    attwp[0:HH, 14:206] = sew24
    attwp[:, 206:238] = ones8
    attwp[:, 238:238 + 4 * 3 * HH] = whhT
    for b in range(BC):
        for mc, mv in ((1774, -9961472.0), (1782, -38400.0),
                       (1790, -128.0)):
            attwp[b, mc + b] = mv
            attwp[b, mc + BC + b] = mv
    return {"wihT": wihT, "_wih2T": wih2T,
            "_bhnr": bhnr, "attwp": attwp, "outw": outw}


def kernel(**inputs):
    if "nc" not in _CACHE:
        _CACHE["nc"] = _build_nc()
    nc, _es = _CACHE["nc"]
    shared = _prep_shared(inputs)
    bf = ml_dtypes.bfloat16
    embp = np.zeros((VOCAB, 256 + E2R), bf)
    embp[:, :E] = np.asarray(inputs["emb"], np.float32).astype(bf)
    embp[0, E] = 1.0  # pad-token indicator column
    inputs = dict(inputs)
    inputs["_embp"] = embp
    in_maps = []
    for c in range(NC):
        m = dict(shared)
        m.update(_prep_core(inputs, c))
        m["brow"] = np.ascontiguousarray(
            np.concatenate([m.pop("_bhnr"), m.pop("_qm0")], axis=1))
        m["q2pack"] = np.ascontiguousarray(
            np.concatenate([m.pop("_epTq2"), m["_wih2T"]], axis=1))
        del m["_wih2T"]
        in_maps.append(m)
    res = run_bass_kernel_spmd(nc, in_maps, list(range(NC)))
    outs = [np.asarray(res.results[c]["out"]) for c in range(NC)]
    se = np.concatenate([o[0:2 * BC].reshape(2, BC, P) for o in outs], axis=1)
    lsm = np.concatenate([o[2 * BC:].reshape(2, BC, P) for o in outs], axis=1)
    return (np.ascontiguousarray(se[0]), np.ascontiguousarray(se[1]),
            np.ascontiguousarray(lsm[0]), np.ascontiguousarray(lsm[1]))



# revision 60
# speedup vs baseline: 1.0158x; 1.0158x over previous
"""AttentionRNN (BiDAF-style QA reader) Trainium2 kernel, v2.

Per core (pure data-parallel over batch, 4 of 32 rows per core):
  1. Host gathers embeddings (padded to 384 rows; column 300 is a pad-token
     indicator that the z-gate weight row turns into a +1e4 logit, freezing
     h across padding exactly, replacing any explicit mask tensor).
  2. xp projections for the 4 GRU directions (bf16 PE) written in
     (gate, step, chunk, batch) layout with warmup pad chunks per region so
     every per-round slice is contiguous; pad-chunk x is memset (BIGM in z)
     so warmup-frozen chains freeze via the same z-pin mechanism.
  3. GRU scan, chunked-parallel: payload chunks of 16 (passage) / 8
     (question) steps, W=10 warmup steps re-run from h=0 (warmup error is
     below the bf16 noise floor).  Two software streams (fwd dirs / bwd
     dirs) run half-a-round offset.  Per round, x slices enter PSUM via
     identity-matmul accumulation; whh @ h' is decomposed by linearity into
     whh@u + whh@v (u = n + z*h, v = -z*n) so the PE input is ready one
     elementwise op after tanh; h' materializes off the critical path and
     payload h' writes the encoder slots directly.
  4. Attention: transposed-logit formulation (softmax row constants w1.p
     and attn_b cancel), per-batch exp on [64,P] tiles with normalization
     deferred through the head matmuls (block-sparse ones lhs accumulates
     per-(head,b) column sums into rows 0:8; se = seA + seB * 1/sums),
     heads via block-sparse lhs weights into rows 0:8, log-softmax tail.
     All PE ops keep partition base 0/32/64 (base 96 and cross-base lhs/rhs
     crash the device).
"""

import contextlib

import numpy as np
import ml_dtypes

import concourse.bass as bass
import concourse.mybir as mybir
from concourse.masks import make_identity
from concourse.tile import TileContext
from concourse.bass_utils import run_bass_kernel_spmd

F32 = mybir.dt.float32
BF16 = mybir.dt.bfloat16
U8 = mybir.dt.uint8
AX = mybir.AxisListType.X
ALU = mybir.AluOpType
AF = mybir.ActivationFunctionType

B, P, Q, E, H, VOCAB = 32, 512, 64, 300, 256, 50000
HH = 128
EPAD = 384
E2 = 64  # third-kc-chunk rows, 45 real (dims 256..299 + indicator) padded
E2R = 45  # real rows in the third chunk
NC = 8
BC = B // NC
NEG = -1e7
BIGM = 1.0e4

import os
SP_ = 16
SQ_ = int(os.environ.get("KSQ", "8"))  # q payload chunk size
NCP, NCQ = P // SP_, Q // SQ_   # 32, 8 chunks
W = int(os.environ.get("KW", "7"))  # warmup rounds (contraction ~0.65/step)
PADP = (W + SP_ - 1) // SP_     # warmup pad chunks
PADQ = (W + SQ_ - 1) // SQ_
CPP, CPQ = NCP + PADP, NCQ + PADQ
RND = W + SP_                # total rounds
NTP, NTQ = BC * P, BC * Q    # 2048, 256

# x tile free-dim strides (elements), layout [128, gate(3), s, cpad, b]
XPG, XPS = SP_ * CPP * BC, CPP * BC     # 2176, 136
XQG, XQS = SQ_ * CPQ * BC, CPQ * BC     # 320, 40
# enc layout [128, s(16), c(40), b(4)]: p chunks 0:32, q chunks 32:40
ENCC = (NCP + NCQ) * BC                 # 160 cols per s-row

_CACHE = {}

V_ACCUM = os.environ.get("KV_ACCUM", "1") == "1"   # exp accum_out in lsm
V_INPLACE = os.environ.get("KV_INPLACE", "0") == "1"  # t1 in-place psum + PE xn
V_SIG3D = os.environ.get("KV_SIG3D", "0") == "1"   # single 3D-AP sigmoid
_PH = int(os.environ.get("KPH", "3"))  # 1=projections 2=+scan 3=full



def _build_nc():
    nc = bass.Bass()

    epTp_d = nc.declare_dram_parameter("epTp_d", [128, 2 * NTP], BF16,
                                       isOutput=False)
    epTp2_d = nc.declare_dram_parameter("epTp2_d", [E2, NTP], BF16,
                                        isOutput=False)
    epTq_d = nc.declare_dram_parameter("epTq_d", [128, 2 * NTQ], BF16,
                                       isOutput=False)
    q2_d = nc.declare_dram_parameter("q2pack", [E2, NTQ + 4 * 3 * HH], BF16,
                                     isOutput=False)
    wihT = nc.declare_dram_parameter("wihT", [128, 4 * 2 * 3 * HH], BF16,
                                     isOutput=False)
    browD = nc.declare_dram_parameter("brow", [1, 576 + NTQ], BF16,
                                      isOutput=False)
    attwD = nc.declare_dram_parameter("attwp", [128, 238 + 4 * 3 * HH + 24],
                                      BF16, isOutput=False)
    outw = nc.declare_dram_parameter("outw", [HH, 4], F32, isOutput=False)
    pm4 = nc.declare_dram_parameter("pm4", [BC, P], BF16, isOutput=False)
    out = nc.declare_dram_parameter("out", [4 * BC, P], F32, isOutput=True)

    es = contextlib.ExitStack()

    def sb(name, shape, dtype):
        return es.enter_context(nc.sbuf_tensor(name, shape, dtype))

    # raw sbuf: written only in the pre-Tile preamble
    epTp = sb("epTp", [128, 2 * NTP], BF16)
    epTp2 = sb("epTp2", [E2, NTP], BF16)
    epTq = sb("epTq", [128, 2 * NTQ], BF16)
    q2_sb = sb("q2_sb", [E2, NTQ + 4 * 3 * HH], BF16)
    wih_sb = sb("wih_sb", [128, 4 * 2 * 3 * HH], BF16)
    brow_sb = sb("brow_sb", [1, 576 + NTQ], BF16)
    attw_sb = sb("attw_sb", [128, 238 + 4 * 3 * HH + 24], BF16)
    outw_sb = sb("outw_sb", [128, 4], F32)
    pm4_sb = sb("pm4_sb", [BC, P], BF16)
    ones_sb = sb("ones_sb", [128, 512], BF16)
    ident_sb = sb("ident_sb", [128, 128], BF16)

    crit_sem = es.enter_context(nc.semaphore("crit_sem"))
    eptp_sem = es.enter_context(nc.semaphore("eptp_sem"))
    auxa_sem = es.enter_context(nc.semaphore("auxa_sem"))
    init_sem = es.enter_context(nc.semaphore("init_sem"))
    shcrit_sem = es.enter_context(nc.semaphore("shcrit_sem"))
    sheptp_sem = es.enter_context(nc.semaphore("sheptp_sem"))

    # ---- pre-tc preamble ----
    # In-tc DRAM->SBUF DMAs crash the exec unit on this stack, and every
    # HWDGE dispatch costs ~650ns of SEQ + shared-HWDGE time, so the load
    # count is pruned to 11 and split across the two HWDGE queues: SP
    # carries only the four q-projection inputs, then relays crit_sem into
    # an in-block shadow inc at ~5.5us; Act carries the rest (tiny packs
    # first, then epTp and the late weights).  Bus order doubles as the
    # ordering guarantee for late consumers: whh and the attention pack
    # transfer right after epTp, several us before PE can reach their
    # first consumer (scan round 1 / attention tail).
    # crit: wih=16 epTq01=32 q2pack=48 | eptp: epTp01=16 epTp2=32
    for _dst, _src in (
            (wih_sb[:, :], wihT[:, :]),
            (epTq[:, :], epTq_d[:, :]),
            (q2_sb[:, :], q2_d[:, :])):
        nc.sync.dma_start(out=_dst, in_=_src).then_inc(crit_sem, 16)
    for _dst, _src, _sem in (
            (brow_sb[:, :], browD[:, :], None),
            (pm4_sb[:, :], pm4[:, :], None),
            (epTp[:, :], epTp_d[:, :], eptp_sem),
            (epTp2[:, :], epTp2_d[:, :], eptp_sem),
            (attw_sb[:, :], attwD[:, :], None),
            (outw_sb[0:HH, :], outw[:, :], None)):
        nc.scalar.dma_start(out=_dst, in_=_src).then_inc(
            _sem if _sem is not None else auxa_sem, 16)

    # identity on gpsimd; constants on vector
    nc.gpsimd.memset(ident_sb[:, :], 0.0)
    nc.gpsimd.affine_select(
        out=ident_sb[:, :], in_=ident_sb[:, :],
        compare_op=ALU.not_equal, fill=1.0, base=0,
        pattern=[[-1, 128]], channel_multiplier=1)
    nc.gpsimd.sem_inc(init_sem, 1)
    nc.vector.memset(ones_sb[:, :], 1.0)
    nc.vector.sem_inc(init_sem, 1)

    # Shadow relays: SP / Pool wait pre-tc for the real DMA sems, then
    # fire an IN-BLOCK shadow inc that the tile scheduler can model; PE
    # waits on the shadows in-block so q-proj starts as soon as wih+epTq
    # land while epTp still streams.  PE's other raw reads (bhnr round 0,
    # whh round 1, sew/sew24/ones8 in the attention tail) ride the bus
    # behind epTp and are resident well before PE can reach them.
    nc.tensor.wait_ge(init_sem, 2)    # ident + ones
    nc.sync.wait_ge(crit_sem, 48)     # relay: q-proj inputs
    nc.gpsimd.wait_ge(eptp_sem, 32)   # relay: epTp chunks

    with TileContext(nc) as tc:
        with tc.tile_pool(name="ps", bufs=2, space="PSUM") as ps, \
             tc.tile_pool(name="sb", bufs=2) as sbp, \
             tc.tile_pool(name="pst", bufs=1) as pst:

            nc.sync.sem_inc(shcrit_sem, 1)
            nc.gpsimd.sem_inc(sheptp_sem, 1)
            _anch = {}

            def pt(name, shape, dtype):
                return pst.tile(shape, dtype, name=name, tag=name)

            ident = ident_sb

            # PE pre-warm: hold the tensor engine's pstate at full clock
            # through the projection phase (ramp: 3us continuous -> 2.4GHz)
            warm = ps.tile([128, 512], F32, name="warm", tag="tq", bufs=1)
            for _ in range(12):
                nc.tensor.matmul(warm[:, :], ident[:, :], ones_sb[:, :],
                                 start=True, stop=True)
            _anch["wc"] = nc.tensor.wait_ge(shcrit_sem, 1)

            xPA = pt("xPA", [128, 3, SP_, CPP, BC], BF16)
            xPB = pt("xPB", [128, 3, SP_, CPP, BC], BF16)
            xQA = pt("xQA", [128, 3, SQ_, CPQ, BC], BF16)
            xQB = pt("xQB", [128, 3, SQ_, CPQ, BC], BF16)
            encA = pt("encA", [128, SP_, ENCC // BC, BC], BF16)
            encB = pt("encB", [128, SP_, ENCC // BC, BC], BF16)
            hA = pt("hA", [128, ENCC], BF16)
            hB = pt("hB", [128, ENCC], BF16)

            pencFB = pt("pencFB", [128, 2 * NTP], BF16)
            qencFB = pt("qencFB", [128, 2 * NTQ], BF16)
            qenc3 = pt("qenc3", [128, 2 * NTQ], BF16)
            qencT = pt("qencT", [64, 8 * HH], BF16)
            qwm = pt("qwm", [1, NTQ], BF16)
            rs8_sb = pt("rs8_sb", [2 * BC, P], F32)
            pawFB = pt("pawFB", [128, 2 * NTP], BF16)
            qsw2 = pt("qsw2", [2, NTQ], BF16)
            qswT8 = pt("qswT8", [64, 32], BF16)
            se8 = pt("se8", [2 * BC, P], F32)
            lsm_sb = pt("lsm_sb", [2 * BC, P], F32)
            lse_sb = pt("lse_sb", [2 * BC, P], F32)
            red_sb = pt("red_sb", [2 * BC, 8], F32)

            nc.vector.memset(hA[:, :], 0)
            nc.vector.memset(hB[:, :], 0)
            nc.vector.memset(qswT8[:, :], 0)
            # x pad chunks: r/n gates -> 0 ; z gate -> BIGM (freeze)
            for xt, np_, c0 in ((xPA, PADP, 0), (xPB, PADP, NCP),
                                (xQA, PADQ, 0), (xQB, PADQ, NCQ)):
                spad = xt[:, :, :, c0:c0 + np_, :]
                nc.vector.memset(spad[:, 0, :, :, :], 0)
                nc.vector.memset(spad[:, 2, :, :, :], 0)
                nc.vector.memset(spad[:, 1, :, :, :], BIGM)

            # ---- projections ----
            # p dirs: per (dir, batch, gate): psum [128,512] = sum_kc wih.T@ep
            # (GPSIMD cannot access PSUM, so moves rotate Act/DVE only)
            mveng = [nc.scalar, nc.vector, nc.gpsimd]
            mvi = 0

            def move(dst, src):
                nonlocal mvi
                eng = mveng[mvi % 2]
                mvi += 1
                if eng is nc.scalar:
                    eng.activation(dst, src, AF.Copy)
                else:
                    eng.tensor_scalar_add(dst, src, 0.0)

            # q dirs projected fully upfront (all q steps feed rounds 0-7),
            # kc-major so matmuls start as soon as each input chunk lands
            for di, (xt, c0) in ((2, (xQA, PADQ)), (3, (xQB, 0))):
                pps = [ps.tile([128, 512], F32, name="pj", tag="pj",
                               bufs=3) for _ in range(3)]
                for kc in range(3):
                    for g in range(3):
                        if kc < 2:
                            wcol = ((di * 2 + kc) * 3 + g) * HH
                            lhs = wih_sb[:, wcol:wcol + HH]
                            rhs = epTq[:, kc * NTQ:(kc + 1) * NTQ]
                        else:
                            wcol = NTQ + (di * 3 + g) * HH
                            lhs = q2_sb[:, wcol:wcol + HH]
                            rhs = q2_sb[:, 0:NTQ]
                        _h = nc.tensor.matmul(pps[g][:, 0:NTQ], lhs, rhs,
                                              start=(kc == 0), stop=(kc == 2))
                        if di == 2 and kc == 0 and g == 0:
                            _anch["qproj"] = _h
                for g in range(3):
                    dst = xt[:, g, :, c0:c0 + NCQ, :] \
                        .rearrange("p s c b -> p b c s")
                    move(dst, pps[g][:, 0:NTQ])

            # p dirs as per-2-step jobs in consumption (pincer) order:
            # stream A consumes s=8..15,0..7; B consumes s=7..0,15..8
            def _pjob(di, xt, c0, sb0, gate=False):
                def f():
                    rhs01 = epTp.rearrange("p (k b c s) -> p k b c s",
                                           k=2, b=BC, s=SP_)
                    rhs2 = epTp2.rearrange("p (b c s) -> p b c s",
                                           b=BC, s=SP_)
                    pps = [ps.tile([128, 512], F32, name="pj", tag="pj",
                                   bufs=3) for _ in range(3)]
                    for kc in range(3):
                        for g in range(3):
                            if kc < 2:
                                wcol = ((di * 2 + kc) * 3 + g) * HH
                                lhs = wih_sb[:, wcol:wcol + HH]
                                rhs = rhs01[:, kc, :, :, sb0:sb0 + 2]
                            else:
                                wcol = NTQ + (di * 3 + g) * HH
                                lhs = q2_sb[:, wcol:wcol + HH]
                                rhs = rhs2[:, :, :, sb0:sb0 + 2]
                            _h = nc.tensor.matmul(
                                pps[g][:, 0:2 * NCP * BC], lhs, rhs,
                                start=(kc == 0), stop=(kc == 2))
                            if gate and kc == 0 and g == 0:
                                _anch["pjob"] = _h
                    for g in range(3):
                        dst = xt[:, g, sb0:sb0 + 2, c0:c0 + NCP, :] \
                            .rearrange("p s c b -> p b c s")
                        move(dst, pps[g][:, 0:2 * NCP * BC])
                return f

            _pjobs = []
            a0 = ((SP_ - W) % SP_) // 2 * 2  # A's first block (even-aligned)
            b0 = ((W - 1) // 2) * 2          # B's first block
            for j in range(8):
                sA = (a0 + 2 * j) % SP_      # A consumes s ascending
                sB = (b0 - 2 * j) % SP_      # B consumes 15-s descending
                _pjobs.append(_pjob(0, xPA, PADP, sA, gate=(j == 0)))
                _pjobs.append(_pjob(1, xPB, 0, sB))
            _anch["we"] = nc.tensor.wait_ge(sheptp_sem, 1)
            for _ in range(2):
                _pjobs.pop(0)()

            if _PH < 2:
                nc.gpsimd.dma_start(out[0:16, :], xPA[0:16, 0, 0:4, PADP:PADP + 32, :])
            # ---- GRU scan ----
            # psum bank layout per stream/round: r[0:160) z[160:320) n[320:480)
            OFR, OFZ, OFN = 0, 160, 320
            SPRM = {"A": (xPA, xQA, encA, hA, 0, 2),
                    "B": (xPB, xQB, encB, hB, 1, 3)}
            _stash = {}

            def geom(st, k):
                e = k - W
                s, coff = e % SP_, e // SP_
                sq, coffq = e % SQ_, e // SQ_
                qact = e < SQ_
                wd = ENCC if qact else NCP * BC
                if st == "A":
                    return e, qact, wd, PADP + coff, PADQ + coffq, s, sq, s
                return (e, qact, wd, -coff, -coffq,
                        SP_ - 1 - s, SQ_ - 1 - sq, SP_ - 1 - s)

            def hsrc_of(st, k):
                xp, xq, enc, hcu, dp, dq = SPRM[st]
                e, qact, wd = geom(st, k)[:3]
                if e - 1 < 0:
                    return hcu[:, 0:wd]
                rprev = (e - 1) if st == "A" else (SP_ - e)
                pcc = ENCC // BC if e - 1 < SQ_ else NCP
                return enc[:, rprev, 0:pcc, :].rearrange(
                    "p c b -> p (c b)")[:, 0:wd]

            def first_half(st, k):
                xp, xq, enc, hcu, dp, dq = SPRM[st]
                e, qact, wd, cp0, cq0, sx, sxq, row = geom(st, k)
                pm = ps.tile([128, 512], F32, name="prz" + st, tag=st)
                mm = nc.tensor.matmul
                first = [True]

                def gmm(dst, lhs, rhs, stop=False):
                    mm(dst, lhs, rhs, start=first[0], stop=stop)
                    first[0] = False

                for g, off in ((0, OFR), (1, OFZ)):
                    gmm(pm[:, off:off + NCP * BC], ident[:, :],
                        xp[:, g, sx, cp0:cp0 + NCP, :])
                    if qact:
                        gmm(pm[:, off + NCP * BC:off + wd], ident[:, :],
                            xq[:, g, sxq, cq0:cq0 + NCQ, :])
                # whh @ h'(k-1) via linearity: h' = u + v (u = n + z*h,
                # v = -z*n); whh@u issues as soon as u is ready, whh@v last.
                if k > 0:
                    for src_t, wtb in zip(_uv[st], (attw_sb, attw_sb)):
                        hp = src_t[:, 0:NCP * BC]
                        hq = src_t[:, NCP * BC:wd] if qact else None
                        for g, off in ((0, OFR), (1, OFZ), (2, OFN)):
                            wc = 238 + (dp * 3 + g) * HH
                            gmm(pm[:, off:off + NCP * BC],
                                wtb[:, wc:wc + HH], hp)
                            if qact:
                                wcq = 238 + (dq * 3 + g) * HH
                                gmm(pm[:, off + NCP * BC:off + wd],
                                    wtb[:, wcq:wcq + HH], hq)
                gmm(pm[:, OFN:OFN + NCP * BC],
                    brow_sb[0:1, dp * HH:(dp + 1) * HH],
                    ones_sb[0:1, 0:NCP * BC], stop=not qact)
                if qact:
                    gmm(pm[:, OFN + NCP * BC:OFN + wd],
                        brow_sb[0:1, dq * HH:(dq + 1) * HH],
                        ones_sb[0:1, 0:NCQ * BC], stop=True)

                # sigmoid r|z in one op
                rz = sbp.tile([128, 2, 160], BF16, name="rz" + st,
                              tag="rz" + st)
                nc.scalar.activation(
                    rz[:, :, 0:wd],
                    pm[:, 0:2 * 160].rearrange(
                        "p (g x) -> p g x", x=160)[:, :, 0:wd],
                    AF.Sigmoid)
                # off-chain: zh = z*h'(k-1)
                zh = sbp.tile([128, 160], BF16, name="zh" + st,
                              tag="zh" + st)
                if k > 0:
                    nc.gpsimd.tensor_mul(zh[:, 0:wd], rz[:, 1, 0:wd],
                                         _hh[st][:, 0:wd])
                # t1 = pn * r in place; += xn via PE identity accumulate
                nc.vector.tensor_mul(pm[:, OFN:OFN + wd],
                                     pm[:, OFN:OFN + wd], rz[:, 0, 0:wd])
                mm(pm[:, OFN:OFN + NCP * BC], ident[:, :],
                   xp[:, 2, sx, cp0:cp0 + NCP, :],
                   start=False, stop=False, skip_group_check=True)
                if qact:
                    mm(pm[:, OFN + NCP * BC:OFN + wd], ident[:, :],
                       xq[:, 2, sxq, cq0:cq0 + NCQ, :],
                       start=False, stop=False, skip_group_check=True)
                _stash[st] = (pm, rz, zh)

            def second_half(st, k):
                xp, xq, enc, hcu, dp, dq = SPRM[st]
                e, qact, wd, cp0, cq0, sx, sxq, row = geom(st, k)
                pm, rz, zh = _stash[st]
                nt = sbp.tile([128, 160], BF16, name="nt" + st,
                              tag="nt" + st)
                nc.scalar.activation(nt[:, 0:wd], pm[:, OFN:OFN + wd],
                                     AF.Tanh)
                # v = -z*n (fused negate: whh@v replaces the old whhN@(z*n))
                # then u = n + zh, back-to-back on DVE
                vt = sbp.tile([128, 160], BF16, name="vt" + st,
                              tag="vt" + st)
                nc.vector.scalar_tensor_tensor(
                    vt[:, 0:wd], rz[:, 1, 0:wd], -1.0, nt[:, 0:wd],
                    op0=ALU.mult, op1=ALU.mult)
                ut = sbp.tile([128, 160], BF16, name="ut" + st,
                              tag="ut" + st)
                if k > 0:
                    nc.vector.tensor_add(ut[:, 0:wd], nt[:, 0:wd],
                                         zh[:, 0:wd])
                else:
                    nc.vector.tensor_copy(ut[:, 0:wd], nt[:, 0:wd])
                _uv[st] = (vt, ut)
                # h' = u + v (off-chain: payload emit / next round's zh)
                if e < 0:
                    hdst = sbp.tile([128, 160], BF16, name="hh" + st,
                                    tag="hh" + st)
                    _hh[st] = hdst
                    hdst = hdst[:, 0:wd]
                else:
                    hdst = enc[:, row, 0:wd // BC, :].rearrange(
                        "p c b -> p (c b)")
                    _hh[st] = hdst
                nc.gpsimd.tensor_add(hdst, ut[:, 0:wd], vt[:, 0:wd])

            _uv = {}
            _hh = {}
            pvv = pencFB.rearrange("p (h b c s) -> p h b c s",
                                   h=2, b=BC, s=SP_)
            _rpk = [nc.vector, nc.gpsimd]

            def _repack_row(half, enc, row, eng):
                # one finalized enc s-row -> pencFB (b,t) layout, one op
                src_ = enc[:, row, 0:NCP, :].rearrange("p c b -> p b c")
                eng.tensor_scalar_add(pvv[:, half, :, :, row], src_, 0.0)
            # q-side attention prep, dribbled into the scan tail
            _qprep = []
            if _PH >= 3:
                def _q_repack(b, half, enc, r0, eng):
                    def f():
                        srcq = enc[:, r0:r0 + SQ_, NCP:NCP + NCQ, b] \
                            .rearrange("p s c -> p c s")
                        dstq = qencFB[:, half * NTQ + b * Q:
                                      half * NTQ + (b + 1) * Q]
                        if eng is nc.scalar:
                            eng.activation(dstq, srcq, AF.Copy)
                        else:
                            eng.tensor_scalar_add(dstq, srcq, 0.0)
                    return f

                qeng = [nc.scalar, nc.vector, nc.gpsimd]
                for b in range(BC):
                    for half, enc, r0 in ((0, encA, 0),
                                          (1, encB, SQ_ % SP_)):
                        _qprep.append(_q_repack(
                            b, half, enc, r0, qeng[(b * 2 + half) % 3]))

                def _qenc3(half):
                    def f():
                        nc.scalar.activation(
                            qenc3[:, half * NTQ:(half + 1) * NTQ],
                            qencFB[:, half * NTQ:(half + 1) * NTQ],
                            AF.Copy, scale=outw_sb[:, half:half + 1])
                    return f
                _qprep.append(_qenc3(0))
                _qprep.append(_qenc3(1))

                def _qwm():
                    pq = ps.tile([128, 512], F32, name="pqw", tag="pj",
                                 bufs=3)
                    nc.tensor.matmul(pq[0:1, 0:NTQ], attw_sb[:, 12:13],
                                     qencFB[:, 0:NTQ], start=True, stop=False)
                    nc.tensor.matmul(pq[0:1, 0:NTQ], attw_sb[:, 13:14],
                                     qencFB[:, NTQ:2 * NTQ], start=False,
                                     stop=True)
                    nc.vector.scalar_tensor_tensor(
                        qwm[0:1, :], brow_sb[0:1, 576:576 + NTQ], NEG, pq[0:1, 0:NTQ],
                        op0=ALU.mult, op1=ALU.add)
                _qprep.append(_qwm)

                def _qsw():
                    # qsw[head, (b,q)] = qenc . sw_attw — folds the attw
                    # head terms into a per-q scalar so seB can consume exT
                    # directly (no attw sbuf bounce)
                    pq2 = ps.tile([128, 512], F32, name="pq2", tag="pj",
                                  bufs=3)
                    nc.tensor.matmul(pq2[0:2, 0:NTQ], attw_sb[:, 4:6],
                                     qencFB[:, 0:NTQ], start=True,
                                     stop=False)
                    nc.tensor.matmul(pq2[0:2, 0:NTQ], attw_sb[:, 6:8],
                                     qencFB[:, NTQ:2 * NTQ], start=False,
                                     stop=True)
                    nc.vector.tensor_scalar_add(qsw2[0:2, :],
                                                pq2[0:2, 0:NTQ], 0.0)
                _qprep.append(_qsw)

                def _qswt(b):
                    def f():
                        ptr = ps.tile([128, 512], BF16, name="ptq",
                                      tag="tq", bufs=1)
                        nc.tensor.transpose(ptr[0:Q, 0:2],
                                            qsw2[0:2, b * Q:(b + 1) * Q],
                                            ident[0:2, 0:2])
                        nc.vector.tensor_scalar_add(
                            qswT8[0:Q, 8 * b + b:8 * b + b + 1],
                            ptr[0:Q, 0:1], 0.0)
                        nc.scalar.activation(
                            qswT8[0:Q, 8 * b + BC + b:8 * b + BC + b + 1],
                            ptr[0:Q, 1:2], AF.Copy)
                    return f
                for b in range(BC):
                    _qprep.append(_qswt(b))

                def _qtr(b, half, eng):
                    def f():
                        ptr = ps.tile([128, 512], BF16, name="ptq", tag="tq",
                                      bufs=1)
                        nc.tensor.transpose(
                            ptr[0:Q, 0:HH],
                            qencFB[:, half * NTQ + b * Q:
                                   half * NTQ + (b + 1) * Q],
                            ident[:, :])
                        col = (b * 2 + half) * HH
                        if eng is nc.scalar:
                            eng.activation(qencT[0:Q, col:col + HH],
                                           ptr[0:Q, 0:HH], AF.Copy)
                        else:
                            eng.tensor_scalar_add(qencT[0:Q, col:col + HH],
                                                  ptr[0:Q, 0:HH], 0.0)
                    return f
                for b in range(BC):
                    for half in range(2):
                        _qprep.append(_qtr(b, half,
                                           qeng[(b * 2 + half) % 2]))

            NRND = RND if _PH >= 2 else 0
            for k in range(NRND):
                first_half("A", k)
                if k > 0:
                    second_half("B", k - 1)
                first_half("B", k)
                second_half("A", k)
                if _PH >= 3 and k >= W:
                    _repack_row(0, encA, k - W, _rpk[k % 2])
                if _PH >= 3 and k > W:
                    _repack_row(1, encB, SP_ - 1 - (k - 1 - W),
                                _rpk[(k + 1) % 2])
                if _pjobs:
                    _pjobs.pop(0)()
                if k == 0 and _pjobs:
                    _pjobs.pop(0)()
                if k > W + SQ_:
                    for _ in range(4):
                        if _qprep:
                            _qprep.pop(0)()
            if NRND:
                second_half("B", NRND - 1)
                if _PH >= 3:
                    _repack_row(1, encB, SP_ - 1 - (NRND - 1 - W),
                                nc.vector)
            while _qprep:
                _qprep.pop(0)()

            if _PH < 3:
                nc.gpsimd.dma_start(out[0:16, :], encA[0:16, 0:4, 0:32, :])

            if _PH >= 3:
                # ---- attention ---- (q-side prep ran in the scan tail)
                # logits read enc directly (strided); no repack barrier
                def pv(enc, b):
                    return enc[:, :, 0:NCP, b].rearrange("p s c -> p c s")

                exT = {}
                for b in range(BC):
                    pt_ = ps.tile([128, 512], F32, name="plgT", tag="A")
                    o = pt_[0:Q, :]
                    nc.tensor.matmul(o, qenc3[:, b * Q:(b + 1) * Q],
                                     pv(encA, b), start=True, stop=False)
                    nc.tensor.matmul(o, qenc3[:, NTQ + b * Q:NTQ + (b + 1) * Q],
                                     pv(encB, b), start=False, stop=False)
                    nc.tensor.matmul(o, qwm[0:1, b * Q:(b + 1) * Q],
                                     ones_sb[0:1, 0:P], start=False, stop=True)
                    ex = sbp.tile([64, 512], BF16, name="exT", tag=f"exT{b}",
                                  bufs=1)
                    nc.scalar.activation(ex[0:Q, :], pt_[0:Q, :], AF.Exp)
                    exT[b] = ex

                # seA head group: penc terms (strided enc reads) with the
                # pad-mask matmul second: the first matmul is enc-gated so
                # the psum allocation stays behind the scan's B-stream
                # banks, and the mask then runs in PE slack, not in the
                # congested seB window.
                seA = ps.tile([128, 512], F32, name="seA", tag="B")
                na = 0
                for b in range(BC):
                    for half, enc in ((0, encA), (1, encB)):
                        blk = (b * 6 + half) * 8
                        nc.tensor.matmul(seA[0:2 * BC, :],
                                         attw_sb[:, 14 + blk:14 + blk + 8],
                                         pv(enc, b), start=(na == 0),
                                         stop=False)
                        na += 1
                        if na == 1:
                            # pad mask: -1e7 = -9961472 - 38400 - 128,
                            # all bf16-exact, so masked rows match the
                            # reference bit-for-bit
                            for mc in (1774, 1782, 1790):
                                nc.tensor.matmul(
                                    seA[0:2 * BC, :],
                                    attw_sb[0:BC, mc:mc + 8],
                                    pm4_sb[0:BC, :], start=False,
                                    stop=False)
                nc.tensor.matmul(seA[0:2 * BC, :], brow_sb[0:1, 512:520],
                                 ones_sb[0:1, 0:P], start=False, stop=True)

                # per-(head,b) column sums into psum rows 0:8 (block-sparse ones)
                sm = ps.tile([128, 512], F32, name="sums", tag="pj", bufs=3)
                for b in range(BC):
                    nc.tensor.matmul(sm[0:2 * BC, :],
                                     attw_sb[0:64, 206 + 8 * b:206 + 8 * b + 8],
                                     exT[b][0:Q, :],
                                     start=(b == 0), stop=(b == BC - 1))
                nc.vector.reciprocal(rs8_sb[:, :], sm[0:2 * BC, :])

                # seB: attw head terms via qswT8 @ exT (no attw bounce),
                # then paw terms; attw_un psum feeds the paw mul directly
                seB = ps.tile([128, 512], F32, name="seB", tag="B")
                for b in range(BC):
                    nc.tensor.matmul(seB[0:2 * BC, :],
                                     qswT8[0:Q, 8 * b:8 * b + 8],
                                     exT[b][0:Q, :],
                                     start=(b == 0), stop=False)
                for b in range(BC):
                    for half in range(2):
                        pw = ps.tile([128, 512], F32, name="paw",
                                     tag="A")
                        col = (b * 2 + half) * HH
                        nc.tensor.matmul(pw[:, :],
                                         qencT[0:Q, col:col + HH],
                                         exT[b][0:Q, :],
                                         start=True, stop=True)
                        sl = slice(half * NTP + b * P,
                                   half * NTP + (b + 1) * P)
                        nc.vector.tensor_mul(pawFB[:, sl],
                                             pencFB[:, sl], pw[:, :])
                        blk = (b * 6 + 4 + half) * 8
                        nc.tensor.matmul(seB[0:2 * BC, :],
                                         attw_sb[:, 14 + blk:14 + blk + 8],
                                         pawFB[:, sl], start=False,
                                         stop=(b == BC - 1 and half == 1))
                # lsm tail: the logits are structurally tiny (|se| <~ 16),
                # so logsumexp runs unshifted — no max reduce on the chain
                t8 = sbp.tile([2 * BC, P], F32, name="t8", tag="t8")
                nc.vector.tensor_mul(t8[:, :], seB[0:2 * BC, :], rs8_sb[:, :])
                nc.vector.tensor_add(se8[:, :], seA[0:2 * BC, :], t8[:, :])
                nc.sync.dma_start(out[0:2 * BC, :], se8[:, :])
                nc.scalar.activation(lse_sb[:, :], se8[:, :], AF.Exp,
                                     accum_out=red_sb[:, 2:3])
                nc.scalar.activation(red_sb[:, 3:4], red_sb[:, 2:3], AF.Ln)
                nc.vector.tensor_scalar(out=lsm_sb[:, :], in0=se8[:, :],
                                        scalar1=red_sb[:, 3:4], scalar2=None,
                                        op0=ALU.subtract)
                nc.sync.dma_start(out[2 * BC:4 * BC, :], lsm_sb[:, :])

    _pin_wait(nc, _anch["wc"], _anch["qproj"])
    _pin_wait(nc, _anch["we"], _anch["pjob"])
    _split_multiwaits(nc)
    return nc, es


def _pin_wait(nc, wait_h, anchor_h):
    """The tile scheduler reorders raw EventSemaphore waits freely; pin
    each one directly before its anchor matmul (and the Ldweights feeding
    it) so the raw-tensor read it protects stays protected."""
    wait_i = getattr(wait_h, "ins", wait_h)
    anchor_i = getattr(anchor_h, "ins", anchor_h)
    for b in nc.main_func.blocks:
        il = b.instructions
        if wait_i in il and anchor_i in il:
            il.remove(wait_i)
            ia = il.index(anchor_i)
            while ia > 0 and type(il[ia - 1]).__name__ == "InstLdweights":
                ia -= 1
            il.insert(ia, wait_i)
            return
    raise AssertionError("pin_wait: wait/anchor not found in one block")


def _split_multiwaits(nc):
    """HW instruction encodings hold a single semaphore wait; move extra
    waits emitted by Tile onto same-engine NOPs inserted just before."""
    for b in nc.main_func.blocks:
        il = b.instructions
        newlist = []
        for inst in il:
            if type(inst).__name__ == "InstISA":
                # EVENT_SEMAPHORE_RANGE_CLEAR mis-encodes for this walrus
                # build; NRT clears semaphores per execution anyway.
                continue
            si = inst.sync_info
            if si is not None and len(si.on_wait) > 1:
                waits = list(si.on_wait)
                for wx in waits[:-1]:
                    nop = nc.engines[inst.engine].nop(hint="wsplit").ins
                    for bb in nc.main_func.blocks:
                        try:
                            bb.instructions.remove(nop)
                            break
                        except ValueError:
                            pass
                    nop.sync_info = mybir.SyncInfo(on_wait=[wx], on_update=[])
                    newlist.append(nop)
                inst.sync_info = mybir.SyncInfo(on_wait=[waits[-1]],
                                                on_update=list(si.on_update))
            newlist.append(inst)
        il[:] = newlist


def _prep_core(inputs, c):
    bs = slice(c * BC, (c + 1) * BC)
    ptok = np.asarray(inputs["passage"][bs]).astype(np.int64).reshape(-1)
    qtok = np.asarray(inputs["question"][bs]).astype(np.int64).reshape(-1)
    d = {}
    embp = inputs["_embp"]  # [VOCAB, 256 + E2R]
    ep = embp[ptok].T       # [256 + E2R, NTP]
    d["epTp_d"] = np.ascontiguousarray(
        ep[0:256].reshape(2, 128, NTP).transpose(1, 0, 2).reshape(128, -1))
    ep2 = np.zeros((E2, NTP), ep.dtype)
    ep2[0:E2R] = ep[256:256 + E2R]
    ep2[E2R] = 1.0  # bias row: pairs with the brzn row folded into wih2T
    d["epTp2_d"] = ep2
    eq = embp[qtok].T
    d["epTq_d"] = np.ascontiguousarray(
        eq[0:256].reshape(2, 128, NTQ).transpose(1, 0, 2).reshape(128, -1))
    eq2 = np.zeros((E2, NTQ), eq.dtype)
    eq2[0:E2R] = eq[256:256 + E2R]
    eq2[E2R] = 1.0
    d["_epTq2"] = eq2
    qm0 = (qtok == 0).astype(ml_dtypes.bfloat16)
    d["_qm0"] = np.ascontiguousarray(qm0[None, :])
    d["pm4"] = np.ascontiguousarray(
        (ptok == 0).reshape(BC, P).astype(ml_dtypes.bfloat16))
    return d


def _prep_shared(inputs):
    bf = ml_dtypes.bfloat16

    wihT = np.zeros((4, 2, 128, 3 * HH), bf)      # (d, kc01, p, m)
    wih2T = np.zeros((4, E2, 3 * HH), bf)         # (d, p2, m)
    whhT = np.zeros((4, HH, 3 * HH), bf)          # (d, p, m)
    brzn = np.zeros((4, HH, 3), np.float32)  # folded into wih2T row E2R
    bhnr = np.zeros((1, 576), bf)  # packed into brow with per-core qm0
    for di, (pre, dd) in enumerate((("p", "f"), ("p", "b"),
                                    ("q", "f"), ("q", "b"))):
        wih = np.asarray(inputs[f"{pre}_wih_{dd}"], np.float32)
        whh = np.asarray(inputs[f"{pre}_whh_{dd}"], np.float32)
        bih = np.asarray(inputs[f"{pre}_bih_{dd}"], np.float32)
        bhh = np.asarray(inputs[f"{pre}_bhh_{dd}"], np.float32)
        wT = np.zeros((EPAD, 3 * HH), bf)
        wT[:E, :] = wih.T.astype(bf)
        wT[E, HH:2 * HH] = BIGM  # pad-token mask column -> z-gate freeze
        wihT[di] = wT[0:256].reshape(2, 128, 3 * HH)
        wih2T[di, 0:E2R] = wT[256:256 + E2R]
        whhT[di] = whh.T.astype(bf)
        for gg in range(3):
            brzn[di, :, gg] = bih[gg * HH:(gg + 1) * HH] + (
                bhh[gg * HH:(gg + 1) * HH] if gg < 2 else 0)
        bhnr[0, di * HH:(di + 1) * HH] = bhh[2 * HH:].astype(bf)
    for di in range(4):
        for gg in range(3):
            wih2T[di, E2R, gg * HH:(gg + 1) * HH] = brzn[di, :, gg].astype(bf)
    wihT = np.ascontiguousarray(
        wihT.transpose(2, 0, 1, 3).reshape(128, -1))      # (p,(d,kc,m))
    wih2T = np.ascontiguousarray(
        wih2T.transpose(1, 0, 2).reshape(E2, -1))         # (p2,(d,m))
    whhT = np.ascontiguousarray(
        whhT.transpose(1, 0, 2).reshape(128, -1))         # (p,(d,m))

    aw = np.asarray(inputs["attn_w"], np.float32)
    w2, w3 = aw[256:512], aw[512:]
    outw = np.zeros((HH, 4), np.float32)
    outw[:, 0], outw[:, 1] = w3[:128], w3[128:]
    outw[0:BC, 2] = float(np.asarray(inputs["start_b"]))
    outw[BC:2 * BC, 2] = float(np.asarray(inputs["end_b"]))

    sw = np.asarray(inputs["start_w"], np.float32)
    ew = np.asarray(inputs["end_w"], np.float32)
    sew = np.zeros((HH, 14), bf)
    for j in range(6):
        sew[:, 2 * j] = sw[j * 128:(j + 1) * 128].astype(bf)
        sew[:, 2 * j + 1] = ew[j * 128:(j + 1) * 128].astype(bf)
    sew[:, 12] = w2[:128].astype(bf)
    sew[:, 13] = w2[128:].astype(bf)
    sew24 = np.zeros((HH, 192), bf)
    for b in range(BC):
        for j in range(6):
            blk = (b * 6 + j) * 8
            sew24[:, blk + b] = sw[j * 128:(j + 1) * 128].astype(bf)
            sew24[:, blk + BC + b] = ew[j * 128:(j + 1) * 128].astype(bf)
    ones8 = np.zeros((128, 32), bf)
    for b in range(BC):
        ones8[:, 8 * b + b] = 1.0
        ones8[:, 8 * b + BC + b] = 1.0
    bhnr[0, 512:512 + BC] = np.float32(inputs["start_b"]).astype(bf)
    bhnr[0, 516:516 + BC] = np.float32(inputs["end_b"]).astype(bf)
    attwp = np.zeros((128, 238 + 4 * 3 * HH + 24), bf)
    attwp[0:HH, 0:14] = sew
    attwp[0:HH, 14:206] = sew24
    attwp[:, 206:238] = ones8
    attwp[:, 238:238 + 4 * 3 * HH] = whhT
    for b in range(BC):
        for mc, mv in ((1774, -9961472.0), (1782, -38400.0),
                       (1790, -128.0)):
            attwp[b, mc + b] = mv
            attwp[b, mc + BC + b] = mv
    return {"wihT": wihT, "_wih2T": wih2T,
            "_bhnr": bhnr, "attwp": attwp, "outw": outw}


def kernel(**inputs):
    if "nc" not in _CACHE:
        _CACHE["nc"] = _build_nc()
    nc, _es = _CACHE["nc"]
    shared = _prep_shared(inputs)
    bf = ml_dtypes.bfloat16
    embp = np.zeros((VOCAB, 256 + E2R), bf)
    embp[:, :E] = np.asarray(inputs["emb"], np.float32).astype(bf)
    embp[0, E] = 1.0  # pad-token indicator column
    inputs = dict(inputs)
    inputs["_embp"] = embp
    in_maps = []
    for c in range(NC):
        m = dict(shared)
        m.update(_prep_core(inputs, c))
        m["brow"] = np.ascontiguousarray(
            np.concatenate([m.pop("_bhnr"), m.pop("_qm0")], axis=1))
        m["q2pack"] = np.ascontiguousarray(
            np.concatenate([m.pop("_epTq2"), m["_wih2T"]], axis=1))
        del m["_wih2T"]
        in_maps.append(m)
    res = run_bass_kernel_spmd(nc, in_maps, list(range(NC)))
    outs = [np.asarray(res.results[c]["out"]) for c in range(NC)]
    se = np.concatenate([o[0:2 * BC].reshape(2, BC, P) for o in outs], axis=1)
    lsm = np.concatenate([o[2 * BC:].reshape(2, BC, P) for o in outs], axis=1)
    return (np.ascontiguousarray(se[0]), np.ascontiguousarray(se[1]),
            np.ascontiguousarray(lsm[0]), np.ascontiguousarray(lsm[1]))

